# revision 40
# baseline (speedup 1.0000x reference)
"""Trainium2 Bass kernel for MultiHeadedAttention (B=4,S=2048,D=1024,H=16).

Sharding: 8 cores = 4 batches x 2 head-groups (8 heads each). No
collectives: each core computes a partial output projection over its 512
attention channels; the host sums the two partials per batch and adds the
bias corrections (bo + Wo@bv).

Layout strategy (everything pre-transposed on host, bf16):
  - qT,kT [ch, s] computed from xT [d, s] with W^T chunks stationary.
  - scores computed TRANSPOSED: scoresT[l, i] = k_h @ q_h^T via row-tiled
    head pairs (K=64 each, tile_position (0,0)/(64,0)).
  - exp fused on ScalarE: exp(raw*0.125 + mask_bias[l]) PSUM->SBUF bf16.
    Mask/padding handled entirely by the per-partition bias column
    (-30000 -> exp == 0), so masked KV rows contribute exactly zero.
  - PV: lhsT = [v_h | ones] (65 cols) stationary, rhs = expT moving;
    row 64 of the accumulator is the softmax denominator Z for free.
  - normalize: 1/Z via DVE reciprocal, partition-broadcast via DMA,
    one DVE multiply writes attnT [ch, s] bf16.
  - out projection: attnT chunks stationary vs Wo^T moving -> [s, m] f32.

KV compaction: positions with mask==0 are dropped on the host before the
K/V projections (exact: reference gives them softmax weight exp(-1e9-max)
== 0.0 in f32). Padded slots get bias -30000.
"""

import sys

for _p in ("/opt/trn_rl_repo", "/root/.axon_site/_ro/trn_rl_repo"):
    if _p not in sys.path:
        sys.path.append(_p)

import numpy as np
import ml_dtypes

B, S, D, H = 4, 2048, 1024, 16
DK = D // H          # 64 head dim
NCORES = 8
HC = H // 2          # 8 heads per core
CH = HC * DK         # 512 channels per core
P = 128
NBLK = 512           # moving free-dim block
VW = 2 * DK          # per-head lhsT block: 64 v cols + 64 ones cols

bf16 = ml_dtypes.bfloat16


def _ceil_to(x, m):
    return ((x + m - 1) // m) * m


def build_nc(SKV, s=S, d=D, hc=HC, postprocess=True):
    """Build the single-core Bass/Tile program (same program for all cores)."""
    import concourse.bass as bass
    import concourse.mybir as mybir
    import concourse.tile as tile

    dt = mybir.dt
    fp32 = dt.float32
    bft = dt.bfloat16
    Exp = mybir.ActivationFunctionType.Exp

    ch = hc * DK
    DC = d // P          # contraction chunks for projections
    CT = ch // P         # channel tiles (128 ch each = 2 heads)
    L = SKV // P         # kv l-tiles
    NQ = s // NBLK       # query blocks
    MBLK = min(NBLK, d)
    MB = d // MBLK       # out-proj output blocks
    SCALE = 1.0 / np.sqrt(np.float32(DK))

    def kvblocks():
        out, b0 = [], 0
        while b0 < SKV:
            bs = min(NBLK, SKV - b0)
            out.append((b0, bs))
            b0 += bs
        return out

    nc = bass.Bass("TRN2", target_bir_lowering=False, debug=False)

    # All staged tensors arrive pre-tiled by the host into the exact SBUF
    # layout ([P, ...] partition-major, chunk orders below), so every
    # staging DMA is a contiguous block copy with multi-KB runs -- the DMA
    # model's throughput collapses with small innermost runs (2.3KB rows
    # gave ~214B/ns, 1KB rows ~100B/ns).
    xqT = nc.dram_tensor("xqT", [P, DC * s], bft, kind="ExternalInput").ap()
    xkT = nc.dram_tensor("xkT", [P, DC * SKV], bft,
                         kind="ExternalInput").ap()
    xvT = nc.dram_tensor("xvT", [P, DC * SKV], bft,
                         kind="ExternalInput").ap()
    wqT = nc.dram_tensor("wqT", [P, DC * ch], bft, kind="ExternalInput").ap()
    wkT = nc.dram_tensor("wkT", [P, DC * ch], bft, kind="ExternalInput").ap()
    wvT = nc.dram_tensor("wvT", [P, DC * ch], bft, kind="ExternalInput").ap()
    woT = nc.dram_tensor("woT", [P, CT * d], bft, kind="ExternalInput").ap()
    bq2 = nc.dram_tensor("bq2", [P, CT], fp32, kind="ExternalInput").ap()
    bk2 = nc.dram_tensor("bk2", [P, CT], fp32, kind="ExternalInput").ap()
    mb2 = nc.dram_tensor("mb2", [P, L], fp32, kind="ExternalInput").ap()
    # bf16 output: halves the output DMA; host combine sums in fp32
    obft = bft
    out = nc.dram_tensor("out", [s, d], obft, kind="ExternalOutput").ap()

    from contextlib import ExitStack

    with tile.TileContext(nc) as tc, ExitStack() as ctx:
        const = ctx.enter_context(tc.tile_pool(name="const", bufs=1))
        psc = ctx.enter_context(tc.tile_pool(name="psc", bufs=2, space="PSUM"))
        pout = ctx.enter_context(tc.tile_pool(name="pout", bufs=2, space="PSUM"))
        pproj = ctx.enter_context(tc.tile_pool(name="pproj", bufs=2, space="PSUM"))
        proj = ctx.enter_context(tc.tile_pool(name="proj", bufs=1))
        expp = ctx.enter_context(tc.tile_pool(name="expp", bufs=12))
        small = ctx.enter_context(tc.tile_pool(name="small", bufs=1))
        obuf = ctx.enter_context(tc.tile_pool(name="obuf", bufs=3))

        # ---- t=0: preload the exp table set + warm the PE HAM clock ------
        jin = const.tile([P, 8], fp32, tag="jin", name="jin")
        jout = const.tile([P, 8], fp32, tag="jout", name="jout")
        nc.vector.memset(jin[:], 0.0)
        nc.scalar.activation(jout[:], jin[:], Exp)
        junk = const.tile([P, NBLK], bft, tag="junk", name="junk")
        nc.vector.memset(junk[:], 0.0)
        for _ in range(14):
            ps = pproj.tile([P, NBLK], fp32, tag="pp", name="ps")
            nc.tensor.matmul(ps[:], lhsT=junk[:, 0:P], rhs=junk[:],
                             start=True, stop=True)

        _ld = [0]

        def load(name, ap, shape, dtp, pool=None, tile_=None, cols=None,
                 eng=None):
            t = tile_ if tile_ is not None else \
                (pool or proj).tile(shape, dtp, tag=name, name=name)
            # Staging triggers go on Sync (k/q path) and GpSimd SWDGE
            # (v path + late xq + wo) — NEVER on Scalar: triggers block
            # their engine on ring credits and the Activation engine must
            # stay free for the exp stream (a scalar-queue trigger backlog
            # stalled the first exp until ~48us).
            eng = eng or nc.sync
            _ld[0] += 1
            if cols is None:
                eng.dma_start(out=t[:], in_=ap)
            else:
                eng.dma_start(out=t[:, cols[0]:cols[1]], in_=ap)
            return t

        # ---- stage inputs; order = consumption order (k, q, v, wo) ------
        # Each dma_start costs ~600ns of issue time on its engine, so the
        # per-dc-chunk staging (59 triggers) was trigger-rate-bound at
        # startup (sync alone spent ~16us just issuing). Fuse every staged
        # tensor into ONE 3D-AP DMA ([(a p) c -> p a c]); consumers slice
        # views of the big tiles. Three parallel queues (~330 GB/s each):
        #   sync:   consts + wk + wq + xq first quarter (3.3 MB, ~16us),
        #           then the output stores
        #   scalar: xk only (2.3 MB, ~13us -- ACT must stay free for exps)
        #   gpsimd: xq q1 (early: qproj(1) fillers pop ~17us), v-path,
        #           xq q2/q3, wo (Pool is otherwise idle)
        bq_sb = load("bq2", bq2[:, :], [P, CT], fp32, const)
        bk_sb = load("bk2", bk2[:, :], [P, CT], fp32, const)
        mb_sb = load("mb2", mb2[:, :], [P, L], fp32, const)

        def bigtile(name, nchunk, width, pool=None):
            t = (pool or proj).tile([P, nchunk * width], bft, tag=name,
                                    name=name)
            return t, [t[:, i * width:(i + 1) * width] for i in range(nchunk)]

        # All DMA queues share one ~460 B/ns HBM port in the model, so the
        # late bulk (v-path, xq q2/q3, wo) must NOT compete with the
        # critical k/q-path during the first ~20us: it is emitted behind a
        # tiny Pool gating copy (see emit_bulk_loads).
        #
        # Host layouts match SBUF exactly, so every load is a contiguous
        # column-slice copy. xk is staged in kv-blocks ((b, dc, c) order on
        # both sides): kproj unit (b0, dc-quad) contracts over all dc but
        # only cols b0..b0+bs, so block-granular sems match consumption.
        wkB, wk_sb = bigtile("wkB", DC, ch)
        nc.sync.dma_start(out=wkB[:, 0:DC // 2 * ch],
                          in_=wkT[:, 0:DC // 2 * ch])
        nc.sync.dma_start(out=wkB[:, DC // 2 * ch:DC * ch],
                          in_=wkT[:, DC // 2 * ch:DC * ch])
        xkB = proj.tile([P, DC * SKV], bft, tag="xkB", name="xkB")
        blocks = kvblocks()
        xkoff = {}
        off = 0
        for (b0, bs) in blocks:
            xkoff[b0] = off
            off += DC * bs

        def xk_load(b0, bs):
            o = xkoff[b0]
            nc.scalar.dma_start(out=xkB[:, o:o + DC * bs],
                                in_=xkT[:, o:o + DC * bs])

        def xk_ap(dc, b0, bs):
            o = xkoff[b0] + dc * bs
            return xkB[:, o:o + bs]

        # Concurrent DMAs on one queue complete nearly TOGETHER (packets
        # interleave), so serialize the critical chain with tiny gating
        # ops: xk block 0 rides the scalar queue ALONE (an ACT exp-gate
        # holds the block-1/2 triggers until it lands), and the q-path
        # waits for wk on the Pool queue.
        xk_load(*blocks[0])
        gjunk = const.tile([P, 8], fp32, tag="gjunk", name="gjunk")
        nc.scalar.activation(gjunk[0:1, :], xkB[0:1, 0:8], Exp)
        for (b0, bs) in blocks[1:]:
            xk_load(b0, bs)

        # q-path: (quarter, dc, c) order; qproj(0) needs wq + quarter 0.
        wqB, wq_sb = bigtile("wqB", DC, ch)
        xqB = proj.tile([P, DC * s], bft, tag="xqB", name="xqB")
        QW = DC * NBLK

        def xq_ap(dc, nq):
            o = nq * QW + dc * NBLK
            return xqB[:, o:o + NBLK]

        gjunk2 = const.tile([P, 8], fp32, tag="gjunk2", name="gjunk2")
        nc.gpsimd.tensor_copy(gjunk2[0:1, :], wkB[0:1, 0:8])
        nc.gpsimd.dma_start(out=wqB[:, :], in_=wqT[:, :])
        nc.gpsimd.dma_start(out=xqB[:, 0:QW], in_=xqT[:, 0:QW])
        wvB, wv_sb = bigtile("wvB", DC, ch)
        xvB, xv_sb = bigtile("xvB", DC, SKV)
        woB, wo_sb = bigtile("woB", CT, d, pool=const)

        def emit_bulk_loads(gate_ap):
            # WAW on xvB[0:1,0:4]: the DMA is ordered after this copy,
            # whose read of gate_ap stalls Pool until the k-path staging
            # has been consumed -- freeing early HBM bandwidth.
            nc.gpsimd.tensor_copy(xvB[0:1, 0:4], gate_ap)
            nc.gpsimd.dma_start(out=xqB[:, QW:2 * QW],
                                in_=xqT[:, QW:2 * QW])
            nc.gpsimd.dma_start(out=wvB[:, :], in_=wvT[:, :])
            nc.gpsimd.dma_start(out=xvB[:, :], in_=xvT[:, :])
            for q in range(2, NQ):
                nc.gpsimd.dma_start(out=xqB[:, q * QW:(q + 1) * QW],
                                    in_=xqT[:, q * QW:(q + 1) * QW])
            nc.gpsimd.dma_start(out=woB[:, :], in_=woT[:, :])

        kT = [const.tile([P, SKV], bft, tag=f"kT{t}", name=f"kT{t}")
              for t in range(CT)]
        vaug = [const.tile([P, hc * VW], bft, tag=f"vaug{l}", name=f"vaug{l}")
                for l in range(L)]

        # ---- K^T projection -> kT [ch, skv] bf16 (+bias per channel) -----
        # emitted as 4-MM units: ct=0 runs ahead of attention, ct 1-3 are
        # popped as PE fillers inside the attention(·, nq=0) l-loops.
        def kproj_units(ct):
            units = []
            for (b0, bs) in kvblocks():
                box = {}

                def mk(b0=b0, bs=bs, dc0=None, box=box):
                    def u():
                        if dc0 == 0:
                            box["ps"] = pproj.tile([P, NBLK], fp32,
                                                   tag="pp", name="ps")
                        ps = box["ps"]
                        for dc in range(dc0, dc0 + 4):
                            nc.tensor.matmul(
                                ps[:, 0:bs], lhsT=wk_sb[dc][:, ct * P:(ct + 1) * P],
                                rhs=xk_ap(dc, b0, bs),
                                start=(dc == 0), stop=(dc == DC - 1),
                                skip_group_check=True)
                        if dc0 + 4 == DC:
                            nc.vector.tensor_scalar_add(
                                kT[ct][:, b0:b0 + bs], ps[:, 0:bs],
                                bk_sb[:, ct:ct + 1])
                    return u

                for dc0 in range(0, DC, 4):
                    units.append(mk(dc0=dc0))
            return units

        # ---- V projection -> vaug [l, (h, 64 v | 64 ones)] bf16 ----------
        def vproj_unit(l):
            def u():
                ps = pproj.tile([P, ch], fp32, tag="pp", name="ps")
                for dc in range(DC):
                    nc.tensor.matmul(
                        ps[:], lhsT=xv_sb[dc][:, l * P:(l + 1) * P],
                        rhs=wv_sb[dc][:],
                        start=(dc == 0), stop=(dc == DC - 1),
                        skip_group_check=True)
                va3 = vaug[l][:].rearrange("p (h w) -> p h w", w=VW)
                ps3 = ps[:].rearrange("p (h k) -> p h k", k=DK)
                nc.vector.tensor_copy(out=va3[:, :, 0:DK], in_=ps3)
                # ones block: the PV matmul broadcasts the softmax
                # denominator Z into output partitions DK..2*DK-1 for free.
                # (DVE, not gpsimd: gpsimd runs the SWDGE staging triggers.)
                nc.vector.memset(va3[:, :, DK:VW], 1.0)
            return u

        # inline only kproj(0)'s first kv-block: attention(0,0)'s qk(0..3)
        # read kT[0][:, 0:512] only, so the stream can start as soon as
        # xk block 0 + wk + wq + xq q0 land (~17us); kproj(0)'s remaining
        # blocks ride the filler queue and land before qk(4) needs them
        k0units = kproj_units(0)
        for u in k0units[0:2]:
            u()

        # ---- per-nq tiles ------------------------------------------------
        qTt = [[const.tile([P, NBLK], bft, tag=f"qT{t}_{q}", name=f"qT{t}_{q}")
                for q in range(NQ)] for t in range(CT)]
        att = [[const.tile([P, NBLK], bft, tag=f"at{t}_{q}", name=f"at{t}_{q}")
                for q in range(NQ)] for t in range(CT)]

        def qt_proj(nq):
            for ct in range(CT):
                ps = pproj.tile([P, NBLK], fp32, tag="pp", name="ps")
                for dc in range(DC):
                    nc.tensor.matmul(
                        ps[:], lhsT=wq_sb[dc][:, ct * P:(ct + 1) * P],
                        rhs=xq_ap(dc, nq),
                        start=(dc == 0), stop=(dc == DC - 1))
                nc.vector.tensor_scalar_add(qTt[ct][nq][:], ps[:],
                                            bq_sb[:, ct:ct + 1])

        # -- PE filler units: small batches of projection work emitted
        #    between attention ops so the PE never starves while ScalarE
        #    streams the exps (the Tile scheduler is greedy by emission
        #    priority; big un-interleaved blobs would run as blobs).
        from collections import deque
        fillers = deque()

        def fill(npop=1):
            for _ in range(npop):
                if fillers:
                    fillers.popleft()()

        def qt_proj_units(nq):
            units = []
            for ct in range(CT):
                box = {}

                def mk(ct=ct, dc0=None, box=box):
                    def u():
                        if dc0 == 0:
                            box["ps"] = pproj.tile([P, NBLK], fp32,
                                                   tag="pp", name="ps")
                        ps = box["ps"]
                        for dc in range(dc0, dc0 + 4):
                            nc.tensor.matmul(
                                ps[:], lhsT=wq_sb[dc][:, ct * P:(ct + 1) * P],
                                rhs=xq_ap(dc, nq),
                                start=(dc == 0), stop=(dc == DC - 1),
                                skip_group_check=True)
                        if dc0 + 4 == DC:
                            nc.vector.tensor_scalar_add(
                                qTt[ct][nq][:], ps[:], bq_sb[:, ct:ct + 1])
                    return u

                for dc0 in range(0, DC, 4):
                    units.append(mk(dc0=dc0))
            return units

        def out_proj_units(nq):
            units = []
            for stl in range(NBLK // P):
                for mbi in range(MB):
                    def mk(stl=stl, mbi=mbi):
                        def u():
                            s0 = nq * NBLK + stl * P
                            m0 = mbi * MBLK
                            ps = pproj.tile([P, MBLK], fp32, tag="pp", name="ps")
                            for ct in range(CT):
                                nc.tensor.matmul(
                                    ps[:],
                                    lhsT=att[ct][nq][:, stl * P:(stl + 1) * P],
                                    rhs=wo_sb[ct][:, m0:m0 + MBLK],
                                    start=(ct == 0), stop=(ct == CT - 1),
                                    skip_group_check=True)
                            ob = obuf.tile([P, MBLK], obft, tag="ob", name="ob")
                            nc.vector.tensor_copy(ob[:], ps[:])
                            # alternate store queues so the final block's
                            # stores drain in parallel at the tail
                            oeng = nc.sync if mbi == 0 else nc.gpsimd
                            oeng.dma_start(out=out[s0:s0 + P, m0:m0 + MBLK],
                                           in_=ob[:])
                        return u
                    units.append(mk())
            return units

        def attention(pr, nq, npop=1, defer=False):
            ops = [pout.tile([P, NBLK], fp32, tag="ops", name="ops")
                   for _ in range(2)]

            def qk(l):
                l0 = l * P
                sp = psc.tile([P, 2 * NBLK], fp32, tag="sp", name="sp")
                # h64 first: Tile attaches the psc WAR wait to the first
                # MM of the pair, so the split-wait EventSemaphore lands
                # BEFORE the pair instead of between the concurrent halves
                for hh in (1, 0):  # head row-tiling within the pair
                    r0 = hh * DK
                    nc.tensor.matmul(
                        sp[:, hh * NBLK:(hh + 1) * NBLK],
                        lhsT=kT[pr][r0:r0 + DK, l0:l0 + P],
                        rhs=qTt[pr][nq][r0:r0 + DK, :],
                        start=True, stop=True, tile_position=(r0, 0))
                e = expp.tile([P, 2 * NBLK], bft, tag="e", name="e")
                nc.scalar.activation(e[:], sp[:], Exp,
                                     bias=mb_sb[:, l:l + 1], scale=SCALE)
                return e

            def pv(l, e):
                for hh in range(2):
                    h = 2 * pr + hh
                    nc.tensor.matmul(
                        ops[hh][:, :],
                        lhsT=vaug[l][:, h * VW:(h + 1) * VW],
                        rhs=e[:, hh * NBLK:(hh + 1) * NBLK],
                        start=(l == 0), stop=(l == L - 1),
                        skip_group_check=True)

            # depth-3 software pipeline: PV(l-3) sits after QK(l) so the
            # exp latency and the previous block's normalize copies are
            # hidden; fillers keep the PE fed while ScalarE runs exps.
            # defer=True (first block only) runs the whole pv-phase after
            # the qk-phase, when vaug lands late during staging; fills
            # continue inside that pv loop so the vproj units (queued to
            # line up with xv's DMA arrival) emit just ahead of the pv
            # that consumes them.
            es = []
            for l in range(L):
                es.append(qk(l))
                fill(npop)
                if not defer and l >= 4:
                    pv(l - 4, es[l - 4])
            for l in range(0 if defer else L - 4, L):
                if defer:
                    fill(npop)
                pv(l, es[l])

            # normalize: stage the PSUM Z rows (partitions DK..2DK hold Z
            # replicated 64x from the ones-columns) into an SBUF tile with
            # matching partition alignment, approx-reciprocal it (18-bit,
            # ample for the bf16 multiply; ~5x cheaper than the 3.3us
            # full-precision DVE reciprocal), then one fused PSUM-read
            # multiply per head writes att bf16 directly -- no separate
            # PSUM->SBUF copy of the attention values.
            a = att[pr][nq]
            zz = small.tile([P, NBLK], fp32, tag="zz", name="zz")
            rz = small.tile([P, NBLK], fp32, tag="rz", name="rz")
            nc.vector.tensor_copy(zz[0:DK, :], ops[0][DK:VW, :])
            nc.vector.tensor_copy(zz[DK:P, :], ops[1][DK:VW, :])
            nc.vector.reciprocal_approx_fast(rz[:, :], zz[:, :])
            nc.vector.tensor_mul(a[0:DK, :], ops[0][0:DK, :], rz[0:DK, :])
            nc.vector.tensor_mul(a[DK:P, :], ops[1][0:DK, :], rz[DK:P, :])

        # ---- main pipeline ----------------------------------------------
        # qproj(0) ct=0 directly; everything else trickles in as fillers
        # between attention ops so no blob head-of-line blocks the PE.
        # attention(0,0) qk-phase pops exactly kproj(1) + q0units[2:8] +
        # kproj(2) (18 units at npop=2); its deferred pv-phase then pops
        # vproj (which needs the full xv, landing ~20us on the gpsimd
        # queue) in lockstep just ahead of each pv, followed by kproj(3).
        q0units = qt_proj_units(0)
        for u in q0units[0:2]:
            u()
        # release the gated bulk loads once the q-path staging has been
        # consumed (qTt[0][0] lands ~19us, written by the units just
        # above) -- after that the HBM port is effectively free
        emit_bulk_loads(qTt[0][0][0:1, 0:4])
        # attention(0,0)'s qk-phase pops exactly 18 units (npop=2 x 9 l):
        # kproj(0) blocks 1-2, kproj(1), q0units[2:8], and the head of
        # kproj(2); its deferred pv-phase then pops vproj (which needs the
        # gated xv, landing ~28us) in lockstep just ahead of each pv,
        # followed by the rest of kproj(2) and kproj(3).
        k2units = kproj_units(2)
        nq0_fill = k0units[2:6] + kproj_units(1) + q0units[2:8]
        nq0_fill += k2units[0:2]
        nq0_fill += [vproj_unit(l) for l in range(L)]
        nq0_fill += k2units[2:6] + kproj_units(3)
        fillers.extend(nq0_fill)
        for nq in range(NQ):
            # interleave next-block Q-proj and prev-block out-proj units
            qunits = qt_proj_units(nq + 1) if nq + 1 < NQ else []
            ounits = out_proj_units(nq - 1) if nq >= 1 else []
            merged, qi, oi = [], 0, 0
            while qi < len(qunits) or oi < len(ounits):
                for _ in range(2):
                    if qi < len(qunits):
                        merged.append(qunits[qi]); qi += 1
                if oi < len(ounits):
                    merged.append(ounits[oi]); oi += 1
            # release the merged fillers per-attention rather than all at
            # once: late blocks have few fillers, and if attention(0, nq)
            # drains them all, the later attentions' qk/pv bursts outrun
            # ScalarE and stall on exp waits -- each such sub-us stall also
            # resets the PE p-state ramp (a ~2-3us hidden tax).
            nmerge = len(merged)
            for pr in range(hc // 2):
                fillers.extend(merged[nmerge * pr // 4:
                                      nmerge * (pr + 1) // 4])
                npop = 2 if nq == 0 else 1
                attention(pr, nq, npop=npop,
                          defer=(nq == 0 and pr == 0))
            while fillers:  # guarantee qTt[·][nq+1] before next block
                fillers.popleft()()
        for u in out_proj_units(NQ - 1):
            u()

    if postprocess:
        _split_mm_waits(nc)
        # Custom-DVE ISA ops (reciprocal_approx_fast) are InstISA
        # subclasses whose .instr bytes are filled by this pass; raw Bass
        # skips it and walrus then fails with "ISA wrong length". Run it
        # after _split_mm_waits so the ISA instructions are sync-free.
        from concourse.library_overlay import lower_extended_insts
        lower_extended_insts(nc)
    return nc


def _split_mm_waits(nc):
    """Walrus's compute-instruction encodings hold a single sync-wait
    command; Tile can emit instructions with 2+ waits ("Too many sync wait
    commands"). Move excess waits onto standalone EventSemaphore ops
    (which hold 2 waits each) inserted just before, on the same engine.
    Queue-based ops (DMA/Drain) tolerate multiple waits and are left."""
    import os
    import bass_rust
    import concourse.mybir as mybir

    limit = int(os.environ.get("SPLIT_LIMIT", "999999"))
    n = 0
    for f in nc.m.functions:
        for blk in f.blocks:
            out = []
            for inst in blk.instructions:
                si = inst.sync_info
                post = None
                if si is not None and inst.opcode != "EventSemaphore":
                    # custom-DVE ISA instructions have a fixed-length
                    # encoding with no room for ANY embedded sync commands:
                    # move waits to a leading EventSemaphore and updates to
                    # a trailing one (engine queues execute in order).
                    is_isa = inst.opcode == "ISA"
                    cap = 0 if is_isa else 1
                    waits = list(si.on_wait or [])
                    upds = list(si.on_update or [])
                    if len(waits) > cap and n < limit:
                        keep = waits[-cap:] if cap else []
                        extra = waits[:-cap] if cap else waits
                        while extra:
                            chunk, extra = extra[:2], extra[2:]
                            n += 1
                            out.append(mybir.InstEventSemaphore(
                                name=f"{inst.name}-evw{n}",
                                engine=inst.engine,
                                ins=[], outs=[],
                                sync_info=bass_rust.SyncInfo(
                                    on_wait=chunk, on_update=[]),
                            ))
                        inst.sync_info = bass_rust.SyncInfo(
                            on_wait=keep, on_update=upds)
                        si = inst.sync_info
                    if is_isa and si.on_update:
                        n += 1
                        post = mybir.InstEventSemaphore(
                            name=f"{inst.name}-evu{n}",
                            engine=inst.engine,
                            ins=[], outs=[],
                            sync_info=bass_rust.SyncInfo(
                                on_wait=[],
                                on_update=list(si.on_update or [])),
                        )
                        inst.sync_info = bass_rust.SyncInfo(
                            on_wait=list(si.on_wait or []), on_update=[])
                out.append(inst)
                if post is not None:
                    out.append(post)
            blk.instructions = out
    return nc


def make_inmaps(query, key, value, mask, Wq, bq, Wk, bk, Wv, bv, Wo, bo):
    """Host-side shard/compact/transpose. Returns (in_maps, SKV)."""
    query = np.asarray(query, np.float32)
    key = np.asarray(key, np.float32)
    value = np.asarray(value, np.float32)
    mask = np.asarray(mask)
    Wq, Wk, Wv, Wo = (np.asarray(w, np.float32) for w in (Wq, Wk, Wv, Wo))
    bq, bk = np.asarray(bq, np.float32), np.asarray(bk, np.float32)

    idxs = []
    for b in range(B):
        idx = np.nonzero(np.asarray(mask[b, 0]) != 0)[0]
        if idx.size == 0:  # degenerate; unreachable for graded inputs
            idx = np.arange(S)
        idxs.append(idx)
    SKV = max(P, _ceil_to(max(len(i) for i in idxs), P))
    L = SKV // P
    CT = CH // P

    def ptile(a):
        # [D0, C] row-major -> [P, (D0//P)*C] with (chunk, col) order per
        # partition: the exact SBUF layout, so staging DMAs are contiguous
        n = a.shape[0] // P
        return np.ascontiguousarray(
            a.reshape(n, P, -1).transpose(1, 0, 2).reshape(P, -1))

    def kvb():
        b0 = 0
        while b0 < SKV:
            bs = min(512, SKV - b0)
            yield b0, bs
            b0 += bs

    per_batch = []
    for b in range(B):
        idx = idxs[b]
        pad = np.zeros(SKV - len(idx), np.int64)
        idx_pad = np.concatenate([idx, pad])
        mbias = np.where(np.arange(SKV) < len(idx), 0.0, -30000.0).astype(np.float32)
        xqt = np.ascontiguousarray(query[b].T).astype(bf16)
        xkt = np.ascontiguousarray(key[b][idx_pad].T).astype(bf16)
        xvt = np.ascontiguousarray(value[b][idx_pad].T).astype(bf16)
        per_batch.append(dict(
            xqT=np.hstack([ptile(xqt[:, q0:q0 + 512])
                           for q0 in range(0, S, 512)]),
            xkT=np.hstack([ptile(xkt[:, b0:b0 + bs]) for b0, bs in kvb()]),
            xvT=ptile(xvt),
            mb2=np.ascontiguousarray(mbias.reshape(L, P).T),
        ))

    in_maps = []
    for c in range(NCORES):
        b, g = divmod(c, 2)
        ch0 = g * CH
        m = dict(per_batch[b])
        m["wqT"] = ptile(Wq[ch0:ch0 + CH].T.astype(bf16))
        m["wkT"] = ptile(Wk[ch0:ch0 + CH].T.astype(bf16))
        m["wvT"] = ptile(Wv[ch0:ch0 + CH].T.astype(bf16))
        m["woT"] = ptile(Wo[:, ch0:ch0 + CH].T.astype(bf16))
        m["bq2"] = np.ascontiguousarray(bq[ch0:ch0 + CH].reshape(CT, P).T)
        m["bk2"] = np.ascontiguousarray(bk[ch0:ch0 + CH].reshape(CT, P).T)
        in_maps.append(m)
    return in_maps, SKV


def combine(results, Wo, bv, bo):
    Wo = np.asarray(Wo, np.float32)
    bv = np.asarray(bv, np.float32)
    bo = np.asarray(bo, np.float32)
    corr = (bo + Wo @ bv).astype(np.float32)
    final = np.empty((B, S, D), np.float32)
    for b in range(B):
        final[b] = (results[2 * b]["out"].astype(np.float32)
                    + results[2 * b + 1]["out"].astype(np.float32)
                    + corr[None, :])
    return final


def kernel(query, key, value, mask, Wq, bq, Wk, bk, Wv, bv, Wo, bo):
    from concourse.bass_utils import run_bass_kernel_spmd

    in_maps, SKV = make_inmaps(query, key, value, mask,
                               Wq, bq, Wk, bk, Wv, bv, Wo, bo)
    nc = build_nc(SKV)
    res = run_bass_kernel_spmd(nc, in_maps, list(range(NCORES)))
    return combine(res.results, Wo, bv, bo)


if __name__ == "__main__":
    rng = np.random.default_rng(0)
    ins = dict(
        query=rng.standard_normal((B, S, D), np.float32),
        key=rng.standard_normal((B, S, D), np.float32),
        value=rng.standard_normal((B, S, D), np.float32),
        mask=(rng.integers(0, 2, (B, 1, S))).astype(np.int32),
        Wq=rng.standard_normal((D, D), np.float32) / 32,
        bq=np.zeros(D, np.float32),
        Wk=rng.standard_normal((D, D), np.float32) / 32,
        bk=np.zeros(D, np.float32),
        Wv=rng.standard_normal((D, D), np.float32) / 32,
        bv=np.zeros(D, np.float32),
        Wo=rng.standard_normal((D, D), np.float32) / 32,
        bo=np.zeros(D, np.float32),
    )
    out = kernel(**ins)
    print("out", out.shape, out.dtype, float(np.abs(out).mean()))



# revision 42
# speedup vs baseline: 1.0044x; 1.0044x over previous
"""Trainium2 Bass kernel for MultiHeadedAttention (B=4,S=2048,D=1024,H=16).

Sharding: 8 cores = 4 batches x 2 head-groups (8 heads each). No
collectives: each core computes a partial output projection over its 512
attention channels; the host sums the two partials per batch and adds the
bias corrections (bo + Wo@bv).

Layout strategy (everything pre-transposed on host, bf16):
  - qT,kT [ch, s] computed from xT [d, s] with W^T chunks stationary.
  - scores computed TRANSPOSED: scoresT[l, i] = k_h @ q_h^T via row-tiled
    head pairs (K=64 each, tile_position (0,0)/(64,0)).
  - exp fused on ScalarE: exp(raw*0.125 + mask_bias[l]) PSUM->SBUF bf16.
    Mask/padding handled entirely by the per-partition bias column
    (-30000 -> exp == 0), so masked KV rows contribute exactly zero.
  - PV: lhsT = [v_h | ones] (65 cols) stationary, rhs = expT moving;
    row 64 of the accumulator is the softmax denominator Z for free.
  - normalize: 1/Z via DVE reciprocal, partition-broadcast via DMA,
    one DVE multiply writes attnT [ch, s] bf16.
  - out projection: attnT chunks stationary vs Wo^T moving -> [s, m] f32.

KV compaction: positions with mask==0 are dropped on the host before the
K/V projections (exact: reference gives them softmax weight exp(-1e9-max)
== 0.0 in f32). Padded slots get bias -30000.
"""

import sys

for _p in ("/opt/trn_rl_repo", "/root/.axon_site/_ro/trn_rl_repo"):
    if _p not in sys.path:
        sys.path.append(_p)

import numpy as np
import ml_dtypes

B, S, D, H = 4, 2048, 1024, 16
DK = D // H          # 64 head dim
NCORES = 8
HC = H // 2          # 8 heads per core
CH = HC * DK         # 512 channels per core
P = 128
NBLK = 512           # moving free-dim block
VW = 2 * DK          # per-head lhsT block: 64 v cols + 64 ones cols

bf16 = ml_dtypes.bfloat16


def _ceil_to(x, m):
    return ((x + m - 1) // m) * m


def build_nc(SKV, s=S, d=D, hc=HC, postprocess=True):
    """Build the single-core Bass/Tile program (same program for all cores)."""
    import concourse.bass as bass
    import concourse.mybir as mybir
    import concourse.tile as tile

    dt = mybir.dt
    fp32 = dt.float32
    bft = dt.bfloat16
    Exp = mybir.ActivationFunctionType.Exp

    ch = hc * DK
    DC = d // P          # contraction chunks for projections
    CT = ch // P         # channel tiles (128 ch each = 2 heads)
    L = SKV // P         # kv l-tiles
    NQ = s // NBLK       # query blocks
    MBLK = min(NBLK, d)
    MB = d // MBLK       # out-proj output blocks
    SCALE = 1.0 / np.sqrt(np.float32(DK))

    def kvblocks():
        out, b0 = [], 0
        while b0 < SKV:
            bs = min(NBLK, SKV - b0)
            out.append((b0, bs))
            b0 += bs
        return out

    nc = bass.Bass("TRN2", target_bir_lowering=False, debug=False)

    # All staged tensors arrive pre-tiled by the host into the exact SBUF
    # layout ([P, ...] partition-major, chunk orders below), so every
    # staging DMA is a contiguous block copy with multi-KB runs -- the DMA
    # model's throughput collapses with small innermost runs (2.3KB rows
    # gave ~214B/ns, 1KB rows ~100B/ns).
    xqT = nc.dram_tensor("xqT", [P, DC * s], bft, kind="ExternalInput").ap()
    xkT = nc.dram_tensor("xkT", [P, DC * SKV], bft,
                         kind="ExternalInput").ap()
    xvT = nc.dram_tensor("xvT", [P, DC * SKV], bft,
                         kind="ExternalInput").ap()
    wqT = nc.dram_tensor("wqT", [P, DC * ch], bft, kind="ExternalInput").ap()
    wkT = nc.dram_tensor("wkT", [P, DC * ch], bft, kind="ExternalInput").ap()
    wvT = nc.dram_tensor("wvT", [P, DC * ch], bft, kind="ExternalInput").ap()
    woT = nc.dram_tensor("woT", [P, CT * d], bft, kind="ExternalInput").ap()
    bq2 = nc.dram_tensor("bq2", [P, CT], fp32, kind="ExternalInput").ap()
    bk2 = nc.dram_tensor("bk2", [P, CT], fp32, kind="ExternalInput").ap()
    mb2 = nc.dram_tensor("mb2", [P, L], fp32, kind="ExternalInput").ap()
    # bf16 output: halves the output DMA; host combine sums in fp32
    obft = bft
    out = nc.dram_tensor("out", [s, d], obft, kind="ExternalOutput").ap()

    from contextlib import ExitStack

    with tile.TileContext(nc) as tc, ExitStack() as ctx:
        const = ctx.enter_context(tc.tile_pool(name="const", bufs=1))
        psc = ctx.enter_context(tc.tile_pool(name="psc", bufs=2, space="PSUM"))
        pout = ctx.enter_context(tc.tile_pool(name="pout", bufs=2, space="PSUM"))
        pproj = ctx.enter_context(tc.tile_pool(name="pproj", bufs=2, space="PSUM"))
        proj = ctx.enter_context(tc.tile_pool(name="proj", bufs=1))
        expp = ctx.enter_context(tc.tile_pool(name="expp", bufs=12))
        small = ctx.enter_context(tc.tile_pool(name="small", bufs=1))
        obuf = ctx.enter_context(tc.tile_pool(name="obuf", bufs=3))

        # ---- t=0: preload the exp table set + warm the PE HAM clock ------
        jin = const.tile([P, 8], fp32, tag="jin", name="jin")
        jout = const.tile([P, 8], fp32, tag="jout", name="jout")
        nc.vector.memset(jin[:], 0.0)
        nc.scalar.activation(jout[:], jin[:], Exp)
        junk = const.tile([P, NBLK], bft, tag="junk", name="junk")
        nc.vector.memset(junk[:], 0.0)
        for _ in range(14):
            ps = pproj.tile([P, NBLK], fp32, tag="pp", name="ps")
            nc.tensor.matmul(ps[:], lhsT=junk[:, 0:P], rhs=junk[:],
                             start=True, stop=True)

        _ld = [0]

        def load(name, ap, shape, dtp, pool=None, tile_=None, cols=None,
                 eng=None):
            t = tile_ if tile_ is not None else \
                (pool or proj).tile(shape, dtp, tag=name, name=name)
            # Staging triggers go on Sync (k/q path) and GpSimd SWDGE
            # (v path + late xq + wo) — NEVER on Scalar: triggers block
            # their engine on ring credits and the Activation engine must
            # stay free for the exp stream (a scalar-queue trigger backlog
            # stalled the first exp until ~48us).
            eng = eng or nc.sync
            _ld[0] += 1
            if cols is None:
                eng.dma_start(out=t[:], in_=ap)
            else:
                eng.dma_start(out=t[:, cols[0]:cols[1]], in_=ap)
            return t

        # ---- stage inputs; order = consumption order (k, q, v, wo) ------
        # Each dma_start costs ~600ns of issue time on its engine, so the
        # per-dc-chunk staging (59 triggers) was trigger-rate-bound at
        # startup (sync alone spent ~16us just issuing). Fuse every staged
        # tensor into ONE 3D-AP DMA ([(a p) c -> p a c]); consumers slice
        # views of the big tiles. Three parallel queues (~330 GB/s each):
        #   sync:   consts + wk + wq + xq first quarter (3.3 MB, ~16us),
        #           then the output stores
        #   scalar: xk only (2.3 MB, ~13us -- ACT must stay free for exps)
        #   gpsimd: xq q1 (early: qproj(1) fillers pop ~17us), v-path,
        #           xq q2/q3, wo (Pool is otherwise idle)
        bq_sb = load("bq2", bq2[:, :], [P, CT], fp32, const)
        bk_sb = load("bk2", bk2[:, :], [P, CT], fp32, const)
        mb_sb = load("mb2", mb2[:, :], [P, L], fp32, const)

        def bigtile(name, nchunk, width, pool=None):
            t = (pool or proj).tile([P, nchunk * width], bft, tag=name,
                                    name=name)
            return t, [t[:, i * width:(i + 1) * width] for i in range(nchunk)]

        # All DMA queues share one ~460 B/ns HBM port in the model, so the
        # late bulk (v-path, xq q2/q3, wo) must NOT compete with the
        # critical k/q-path during the first ~20us: it is emitted behind a
        # tiny Pool gating copy (see emit_bulk_loads).
        #
        # Host layouts match SBUF exactly, so every load is a contiguous
        # column-slice copy. xk is staged in kv-blocks ((b, dc, c) order on
        # both sides): kproj unit (b0, dc-quad) contracts over all dc but
        # only cols b0..b0+bs, so block-granular sems match consumption.
        wkB, wk_sb = bigtile("wkB", DC, ch)
        nc.sync.dma_start(out=wkB[:, 0:DC // 2 * ch],
                          in_=wkT[:, 0:DC // 2 * ch])
        nc.sync.dma_start(out=wkB[:, DC // 2 * ch:DC * ch],
                          in_=wkT[:, DC // 2 * ch:DC * ch])
        xkB = proj.tile([P, DC * SKV], bft, tag="xkB", name="xkB")
        blocks = kvblocks()
        xkoff = {}
        off = 0
        for (b0, bs) in blocks:
            xkoff[b0] = off
            off += DC * bs

        def xk_load(b0, bs):
            o = xkoff[b0]
            nc.scalar.dma_start(out=xkB[:, o:o + DC * bs],
                                in_=xkT[:, o:o + DC * bs])

        def xk_ap(dc, b0, bs):
            o = xkoff[b0] + dc * bs
            return xkB[:, o:o + bs]

        # Concurrent DMAs on one queue complete nearly TOGETHER (packets
        # interleave), so serialize the critical chain with tiny gating
        # ops: xk block 0 rides the scalar queue ALONE (an ACT exp-gate
        # holds the block-1/2 triggers until it lands), and the q-path
        # waits for wk on the Pool queue.
        xk_load(*blocks[0])
        gjunk = const.tile([P, 8], fp32, tag="gjunk", name="gjunk")
        nc.scalar.activation(gjunk[0:1, :], xkB[0:1, 0:8], Exp)
        for (b0, bs) in blocks[1:]:
            xk_load(b0, bs)

        # q-path: (quarter, dc, c) order; qproj(0) needs wq + quarter 0.
        wqB, wq_sb = bigtile("wqB", DC, ch)
        xqB = proj.tile([P, DC * s], bft, tag="xqB", name="xqB")
        QW = DC * NBLK

        def xq_ap(dc, nq):
            o = nq * QW + dc * NBLK
            return xqB[:, o:o + NBLK]

        gjunk2 = const.tile([P, 8], fp32, tag="gjunk2", name="gjunk2")
        nc.gpsimd.tensor_copy(gjunk2[0:1, :], wkB[0:1, 0:8])
        nc.gpsimd.dma_start(out=wqB[:, :], in_=wqT[:, :])
        nc.gpsimd.dma_start(out=xqB[:, 0:QW], in_=xqT[:, 0:QW])
        wvB, wv_sb = bigtile("wvB", DC, ch)
        xvB, xv_sb = bigtile("xvB", DC, SKV)
        woB, wo_sb = bigtile("woB", CT, d, pool=const)

        def emit_bulk_loads(gate_ap):
            # WAW on xvB[0:1,0:4]: the DMA is ordered after this copy,
            # whose read of gate_ap stalls Pool until the k-path staging
            # has been consumed -- freeing early HBM bandwidth.
            nc.gpsimd.tensor_copy(xvB[0:1, 0:4], gate_ap)
            nc.gpsimd.dma_start(out=xqB[:, QW:2 * QW],
                                in_=xqT[:, QW:2 * QW])
            nc.gpsimd.dma_start(out=wvB[:, :], in_=wvT[:, :])
            nc.gpsimd.dma_start(out=xvB[:, :], in_=xvT[:, :])
            for q in range(2, NQ):
                nc.gpsimd.dma_start(out=xqB[:, q * QW:(q + 1) * QW],
                                    in_=xqT[:, q * QW:(q + 1) * QW])
            nc.gpsimd.dma_start(out=woB[:, :], in_=woT[:, :])

        kT = [const.tile([P, SKV], bft, tag=f"kT{t}", name=f"kT{t}")
              for t in range(CT)]
        vaug = [const.tile([P, hc * VW], bft, tag=f"vaug{l}", name=f"vaug{l}")
                for l in range(L)]

        # ---- K^T projection -> kT [ch, skv] bf16 (+bias per channel) -----
        # emitted as 4-MM units: ct=0 runs ahead of attention, ct 1-3 are
        # popped as PE fillers inside the attention(·, nq=0) l-loops.
        def kproj_units(ct):
            units = []
            for (b0, bs) in kvblocks():
                box = {}

                def mk(b0=b0, bs=bs, dc0=None, box=box):
                    def u():
                        if dc0 == 0:
                            box["ps"] = pproj.tile([P, NBLK], fp32,
                                                   tag="pp", name="ps")
                        ps = box["ps"]
                        for dc in range(dc0, dc0 + 4):
                            nc.tensor.matmul(
                                ps[:, 0:bs], lhsT=wk_sb[dc][:, ct * P:(ct + 1) * P],
                                rhs=xk_ap(dc, b0, bs),
                                start=(dc == 0), stop=(dc == DC - 1),
                                skip_group_check=True)
                        if dc0 + 4 == DC:
                            nc.vector.tensor_scalar_add(
                                kT[ct][:, b0:b0 + bs], ps[:, 0:bs],
                                bk_sb[:, ct:ct + 1])
                    return u

                for dc0 in range(0, DC, 4):
                    units.append(mk(dc0=dc0))
            return units

        # ---- V projection -> vaug [l, (h, 64 v | 64 ones)] bf16 ----------
        def vproj_unit(l):
            def u():
                ps = pproj.tile([P, ch], fp32, tag="pp", name="ps")
                for dc in range(DC):
                    nc.tensor.matmul(
                        ps[:], lhsT=xv_sb[dc][:, l * P:(l + 1) * P],
                        rhs=wv_sb[dc][:],
                        start=(dc == 0), stop=(dc == DC - 1),
                        skip_group_check=True)
                va3 = vaug[l][:].rearrange("p (h w) -> p h w", w=VW)
                ps3 = ps[:].rearrange("p (h k) -> p h k", k=DK)
                nc.vector.tensor_copy(out=va3[:, :, 0:DK], in_=ps3)
                # ones block: the PV matmul broadcasts the softmax
                # denominator Z into output partitions DK..2*DK-1 for free.
                # (DVE, not gpsimd: gpsimd runs the SWDGE staging triggers.)
                nc.vector.memset(va3[:, :, DK:VW], 1.0)
            return u

        # inline only kproj(0)'s first kv-block: attention(0,0)'s qk(0..3)
        # read kT[0][:, 0:512] only, so the stream can start as soon as
        # xk block 0 + wk + wq + xq q0 land (~17us); kproj(0)'s remaining
        # blocks ride the filler queue and land before qk(4) needs them
        k0units = kproj_units(0)
        for u in k0units[0:2]:
            u()

        # ---- per-nq tiles ------------------------------------------------
        qTt = [[const.tile([P, NBLK], bft, tag=f"qT{t}_{q}", name=f"qT{t}_{q}")
                for q in range(NQ)] for t in range(CT)]
        att = [[const.tile([P, NBLK], bft, tag=f"at{t}_{q}", name=f"at{t}_{q}")
                for q in range(NQ)] for t in range(CT)]

        def qt_proj(nq):
            for ct in range(CT):
                ps = pproj.tile([P, NBLK], fp32, tag="pp", name="ps")
                for dc in range(DC):
                    nc.tensor.matmul(
                        ps[:], lhsT=wq_sb[dc][:, ct * P:(ct + 1) * P],
                        rhs=xq_ap(dc, nq),
                        start=(dc == 0), stop=(dc == DC - 1))
                nc.vector.tensor_scalar_add(qTt[ct][nq][:], ps[:],
                                            bq_sb[:, ct:ct + 1])

        # -- PE filler units: small batches of projection work emitted
        #    between attention ops so the PE never starves while ScalarE
        #    streams the exps (the Tile scheduler is greedy by emission
        #    priority; big un-interleaved blobs would run as blobs).
        from collections import deque
        fillers = deque()

        def fill(npop=1):
            for _ in range(npop):
                if fillers:
                    fillers.popleft()()

        def qt_proj_units(nq):
            units = []
            for ct in range(CT):
                box = {}

                def mk(ct=ct, dc0=None, box=box):
                    def u():
                        if dc0 == 0:
                            box["ps"] = pproj.tile([P, NBLK], fp32,
                                                   tag="pp", name="ps")
                        ps = box["ps"]
                        for dc in range(dc0, dc0 + 4):
                            nc.tensor.matmul(
                                ps[:], lhsT=wq_sb[dc][:, ct * P:(ct + 1) * P],
                                rhs=xq_ap(dc, nq),
                                start=(dc == 0), stop=(dc == DC - 1),
                                skip_group_check=True)
                        if dc0 + 4 == DC:
                            nc.vector.tensor_scalar_add(
                                qTt[ct][nq][:], ps[:], bq_sb[:, ct:ct + 1])
                    return u

                for dc0 in range(0, DC, 4):
                    units.append(mk(dc0=dc0))
            return units

        def out_proj_units(nq):
            units = []
            for stl in range(NBLK // P):
                for mbi in range(MB):
                    def mk(stl=stl, mbi=mbi):
                        def u():
                            s0 = nq * NBLK + stl * P
                            m0 = mbi * MBLK
                            ps = pproj.tile([P, MBLK], fp32, tag="pp", name="ps")
                            for ct in range(CT):
                                nc.tensor.matmul(
                                    ps[:],
                                    lhsT=att[ct][nq][:, stl * P:(stl + 1) * P],
                                    rhs=wo_sb[ct][:, m0:m0 + MBLK],
                                    start=(ct == 0), stop=(ct == CT - 1),
                                    skip_group_check=True)
                            ob = obuf.tile([P, MBLK], obft, tag="ob", name="ob")
                            nc.vector.tensor_copy(ob[:], ps[:])
                            # alternate store queues so the final block's
                            # stores drain in parallel at the tail
                            oeng = nc.sync if mbi == 0 else nc.gpsimd
                            oeng.dma_start(out=out[s0:s0 + P, m0:m0 + MBLK],
                                           in_=ob[:])
                        return u
                    units.append(mk())
            return units

        def attention(pr, nq, npop=1, defer=False):
            ops = [pout.tile([P, NBLK], fp32, tag="ops", name="ops")
                   for _ in range(2)]

            def qk(l):
                l0 = l * P
                sp = psc.tile([P, 2 * NBLK], fp32, tag="sp", name="sp")
                # h64 first: Tile attaches the psc WAR wait to the first
                # MM of the pair, so the split-wait EventSemaphore lands
                # BEFORE the pair instead of between the concurrent halves
                for hh in (1, 0):  # head row-tiling within the pair
                    r0 = hh * DK
                    nc.tensor.matmul(
                        sp[:, hh * NBLK:(hh + 1) * NBLK],
                        lhsT=kT[pr][r0:r0 + DK, l0:l0 + P],
                        rhs=qTt[pr][nq][r0:r0 + DK, :],
                        start=True, stop=True, tile_position=(r0, 0))
                e = expp.tile([P, 2 * NBLK], bft, tag="e", name="e")
                nc.scalar.activation(e[:], sp[:], Exp,
                                     bias=mb_sb[:, l:l + 1], scale=SCALE)
                return e

            def pv(l, e):
                for hh in range(2):
                    h = 2 * pr + hh
                    nc.tensor.matmul(
                        ops[hh][:, :],
                        lhsT=vaug[l][:, h * VW:(h + 1) * VW],
                        rhs=e[:, hh * NBLK:(hh + 1) * NBLK],
                        start=(l == 0), stop=(l == L - 1),
                        skip_group_check=True)

            # depth-3 software pipeline: PV(l-3) sits after QK(l) so the
            # exp latency and the previous block's normalize copies are
            # hidden; fillers keep the PE fed while ScalarE runs exps.
            # defer=True (first block only) runs the whole pv-phase after
            # the qk-phase, when vaug lands late during staging; fills
            # continue inside that pv loop so the vproj units (queued to
            # line up with xv's DMA arrival) emit just ahead of the pv
            # that consumes them.
            es = []
            for l in range(L):
                es.append(qk(l))
                fill(npop)
                if not defer and l >= 4:
                    pv(l - 4, es[l - 4])
            for l in range(0 if defer else L - 4, L):
                if defer:
                    fill(npop)
                pv(l, es[l])

            # normalize: stage the PSUM Z rows (partitions DK..2DK hold Z
            # replicated 64x from the ones-columns) into an SBUF tile with
            # matching partition alignment, approx-reciprocal it (18-bit,
            # ample for the bf16 multiply; ~5x cheaper than the 3.3us
            # full-precision DVE reciprocal), then one fused PSUM-read
            # multiply per head writes att bf16 directly -- no separate
            # PSUM->SBUF copy of the attention values.
            a = att[pr][nq]
            zz = small.tile([P, NBLK], fp32, tag="zz", name="zz")
            rz = small.tile([P, NBLK], fp32, tag="rz", name="rz")
            nc.vector.tensor_copy(zz[0:DK, :], ops[0][DK:VW, :])
            nc.vector.tensor_copy(zz[DK:P, :], ops[1][DK:VW, :])
            nc.vector.reciprocal_approx_fast(rz[:, :], zz[:, :])
            nc.vector.tensor_mul(a[0:DK, :], ops[0][0:DK, :], rz[0:DK, :])
            nc.vector.tensor_mul(a[DK:P, :], ops[1][0:DK, :], rz[DK:P, :])

        # ---- main pipeline ----------------------------------------------
        # qproj(0) ct=0 directly; everything else trickles in as fillers
        # between attention ops so no blob head-of-line blocks the PE.
        # attention(0,0) qk-phase pops exactly kproj(1) + q0units[2:8] +
        # kproj(2) (18 units at npop=2); its deferred pv-phase then pops
        # vproj (which needs the full xv, landing ~20us on the gpsimd
        # queue) in lockstep just ahead of each pv, followed by kproj(3).
        q0units = qt_proj_units(0)
        for u in q0units[0:2]:
            u()
        # release the gated bulk loads once the q-path staging has been
        # consumed (qTt[0][0] lands ~19us, written by the units just
        # above) -- after that the HBM port is effectively free
        emit_bulk_loads(qTt[0][0][0:1, 0:4])
        # attention(0,0)'s qk-phase pops exactly 18 units (npop=2 x 9 l):
        # kproj(0) blocks 1-2, kproj(1), q0units[2:8], and the head of
        # kproj(2); its deferred pv-phase then pops vproj (which needs the
        # gated xv, landing ~28us) in lockstep just ahead of each pv,
        # followed by the rest of kproj(2) and kproj(3).
        k2units = kproj_units(2)
        nq0_fill = k0units[2:6] + kproj_units(1) + q0units[2:8]
        nq0_fill += k2units[0:2]
        nq0_fill += [vproj_unit(l) for l in range(L)]
        nq0_fill += k2units[2:6] + kproj_units(3)
        fillers.extend(nq0_fill)
        for nq in range(NQ):
            # interleave next-block Q-proj and prev-block out-proj units
            qunits = qt_proj_units(nq + 1) if nq + 1 < NQ else []
            ounits = out_proj_units(nq - 1) if nq >= 1 else []
            merged, qi, oi = [], 0, 0
            while qi < len(qunits) or oi < len(ounits):
                for _ in range(2):
                    if qi < len(qunits):
                        merged.append(qunits[qi]); qi += 1
                if oi < len(ounits):
                    merged.append(ounits[oi]); oi += 1
            # release the merged fillers per-attention rather than all at
            # once: late blocks have few fillers, and if attention(0, nq)
            # drains them all, the later attentions' qk/pv bursts outrun
            # ScalarE and stall on exp waits -- each such sub-us stall also
            # resets the PE p-state ramp (a ~2-3us hidden tax).
            nmerge = len(merged)
            for pr in range(hc // 2):
                fillers.extend(merged[nmerge * pr // 4:
                                      nmerge * (pr + 1) // 4])
                npop = 2 if nq == 0 else 1
                attention(pr, nq, npop=npop,
                          defer=(nq == 0 and pr == 0))
            while fillers:  # guarantee qTt[·][nq+1] before next block
                fillers.popleft()()
        for u in out_proj_units(NQ - 1):
            u()

    if postprocess:
        _split_mm_waits(nc)
        # Custom-DVE ISA ops (reciprocal_approx_fast) are InstISA
        # subclasses whose .instr bytes are filled by this pass; raw Bass
        # skips it and walrus then fails with "ISA wrong length". Run it
        # after _split_mm_waits so the ISA instructions are sync-free.
        from concourse.library_overlay import lower_extended_insts
        lower_extended_insts(nc)
    return nc


def _split_mm_waits(nc):
    """Walrus's compute-instruction encodings hold a single sync-wait
    command; Tile can emit instructions with 2+ waits ("Too many sync wait
    commands"). Move excess waits onto standalone EventSemaphore ops
    (which hold 2 waits each) inserted just before, on the same engine.
    Queue-based ops (DMA/Drain) tolerate multiple waits and are left."""
    import os
    import bass_rust
    import concourse.mybir as mybir

    limit = int(os.environ.get("SPLIT_LIMIT", "999999"))
    n = 0
    for f in nc.m.functions:
        for blk in f.blocks:
            out = []
            for inst in blk.instructions:
                si = inst.sync_info
                post = None
                if si is not None and inst.opcode != "EventSemaphore":
                    # custom-DVE ISA instructions have a fixed-length
                    # encoding with no room for ANY embedded sync commands:
                    # move waits to a leading EventSemaphore and updates to
                    # a trailing one (engine queues execute in order).
                    is_isa = inst.opcode == "ISA"
                    cap = 0 if is_isa else 1
                    waits = list(si.on_wait or [])
                    upds = list(si.on_update or [])
                    if len(waits) > cap and n < limit:
                        keep = waits[-cap:] if cap else []
                        extra = waits[:-cap] if cap else waits
                        while extra:
                            chunk, extra = extra[:2], extra[2:]
                            n += 1
                            out.append(mybir.InstEventSemaphore(
                                name=f"{inst.name}-evw{n}",
                                engine=inst.engine,
                                ins=[], outs=[],
                                sync_info=bass_rust.SyncInfo(
                                    on_wait=chunk, on_update=[]),
                            ))
                        inst.sync_info = bass_rust.SyncInfo(
                            on_wait=keep, on_update=upds)
                        si = inst.sync_info
                    if is_isa and si.on_update:
                        n += 1
                        post = mybir.InstEventSemaphore(
                            name=f"{inst.name}-evu{n}",
                            engine=inst.engine,
                            ins=[], outs=[],
                            sync_info=bass_rust.SyncInfo(
                                on_wait=[],
                                on_update=list(si.on_update or [])),
                        )
                        inst.sync_info = bass_rust.SyncInfo(
                            on_wait=list(si.on_wait or []), on_update=[])
                out.append(inst)
                if post is not None:
                    out.append(post)
            blk.instructions = out
    return nc


def make_inmaps(query, key, value, mask, Wq, bq, Wk, bk, Wv, bv, Wo, bo):
    """Host-side shard/compact/transpose. Returns (in_maps, SKV)."""
    query = np.asarray(query, np.float32)
    key = np.asarray(key, np.float32)
    value = np.asarray(value, np.float32)
    mask = np.asarray(mask)
    Wq, Wk, Wv, Wo = (np.asarray(w, np.float32) for w in (Wq, Wk, Wv, Wo))
    bq, bk = np.asarray(bq, np.float32), np.asarray(bk, np.float32)

    idxs = []
    for b in range(B):
        idx = np.nonzero(np.asarray(mask[b, 0]) != 0)[0]
        if idx.size == 0:  # degenerate; unreachable for graded inputs
            idx = np.arange(S)
        idxs.append(idx)
    SKV = max(P, _ceil_to(max(len(i) for i in idxs), P))
    L = SKV // P
    CT = CH // P

    def ptile(a):
        # [D0, C] row-major -> [P, (D0//P)*C] with (chunk, col) order per
        # partition: the exact SBUF layout, so staging DMAs are contiguous
        n = a.shape[0] // P
        return np.ascontiguousarray(
            a.reshape(n, P, -1).transpose(1, 0, 2).reshape(P, -1))

    def kvb():
        b0 = 0
        while b0 < SKV:
            bs = min(512, SKV - b0)
            yield b0, bs
            b0 += bs

    per_batch = []
    for b in range(B):
        idx = idxs[b]
        pad = np.zeros(SKV - len(idx), np.int64)
        idx_pad = np.concatenate([idx, pad])
        mbias = np.where(np.arange(SKV) < len(idx), 0.0, -30000.0).astype(np.float32)
        xqt = np.ascontiguousarray(query[b].T).astype(bf16)
        xkt = np.ascontiguousarray(key[b][idx_pad].T).astype(bf16)
        xvt = np.ascontiguousarray(value[b][idx_pad].T).astype(bf16)
        per_batch.append(dict(
            xqT=np.hstack([ptile(xqt[:, q0:q0 + 512])
                           for q0 in range(0, S, 512)]),
            xkT=np.hstack([ptile(xkt[:, b0:b0 + bs]) for b0, bs in kvb()]),
            xvT=ptile(xvt),
            mb2=np.ascontiguousarray(mbias.reshape(L, P).T),
        ))

    in_maps = []
    for c in range(NCORES):
        b, g = divmod(c, 2)
        ch0 = g * CH
        m = dict(per_batch[b])
        m["wqT"] = ptile(Wq[ch0:ch0 + CH].T.astype(bf16))
        m["wkT"] = ptile(Wk[ch0:ch0 + CH].T.astype(bf16))
        m["wvT"] = ptile(Wv[ch0:ch0 + CH].T.astype(bf16))
        m["woT"] = ptile(Wo[:, ch0:ch0 + CH].T.astype(bf16))
        m["bq2"] = np.ascontiguousarray(bq[ch0:ch0 + CH].reshape(CT, P).T)
        m["bk2"] = np.ascontiguousarray(bk[ch0:ch0 + CH].reshape(CT, P).T)
        in_maps.append(m)
    return in_maps, SKV


def combine(results, Wo, bv, bo):
    Wo = np.asarray(Wo, np.float32)
    bv = np.asarray(bv, np.float32)
    bo = np.asarray(bo, np.float32)
    corr = (bo + Wo @ bv).astype(np.float32)
    final = np.empty((B, S, D), np.float32)
    for b in range(B):
        final[b] = (results[2 * b]["out"].astype(np.float32)
                    + results[2 * b + 1]["out"].astype(np.float32)
                    + corr[None, :])
    return final


def kernel(query, key, value, mask, Wq, bq, Wk, bk, Wv, bv, Wo, bo):
    from concourse.bass_utils import run_bass_kernel_spmd

    in_maps, SKV = make_inmaps(query, key, value, mask,
                               Wq, bq, Wk, bk, Wv, bv, Wo, bo)
    nc = build_nc(SKV)
    res = run_bass_kernel_spmd(nc, in_maps, list(range(NCORES)))
    return combine(res.results, Wo, bv, bo)


if __name__ == "__main__":
    rng = np.random.default_rng(0)
    ins = dict(
        query=rng.standard_normal((B, S, D), np.float32),
        key=rng.standard_normal((B, S, D), np.float32),
        value=rng.standard_normal((B, S, D), np.float32),
        mask=(rng.integers(0, 2, (B, 1, S))).astype(np.int32),
        Wq=rng.standard_normal((D, D), np.float32) / 32,
        bq=np.zeros(D, np.float32),
        Wk=rng.standard_normal((D, D), np.float32) / 32,
        bk=np.zeros(D, np.float32),
        Wv=rng.standard_normal((D, D), np.float32) / 32,
        bv=np.zeros(D, np.float32),
        Wo=rng.standard_normal((D, D), np.float32) / 32,
        bo=np.zeros(D, np.float32),
    )
    out = kernel(**ins)
    print("out", out.shape, out.dtype, float(np.abs(out).mean()))

